# revision 31
# baseline (speedup 1.0000x reference)
"""DenseAtt kernel for Trainium2 (Bass/Tile), 8-core batch-parallel.

Math (per batch element b):
    s_left  = x @ W[:d]          # [n]
    s_right = x @ W[d:]          # [n]
    out[i,j] = sigmoid(s_left[i] + s_right[j] + bias) * adj[i,j]

Shapes: x [8, 2048, 128] f32, adj [8, 2048, 2048] f32, W [256] f32, b [] f32.
Sharding: one batch element per NeuronCore (B == n_cores == 8), no
collectives; full inputs in, full output out, gather on host.

The kernel is HBM-bound (per core: adj in + out back). Both streams — and
x/W — move as bfloat16 (host converts f32->bf16, upcasts out bf16->f32);
max rel err vs the f32 reference is 1.31e-2 (host + CoreSim + HW
validated), inside the 2e-2 gate, and main-loop HBM traffic halves:
32MB -> 16MB per core. TimelineSim: 51786 ns vs 113650 ns for the f32
baseline; DMA engines run 100% busy from first byte to last.

Device plan per core:
  1. Host passes xT = x.T bf16 (no on-device transposes) plus wc [128,2]
     bf16 (w_l|w_r columns) and bb [128,1] = b f32.
  2. PE: sr_b = w_r_bcast.T @ xT -> four [128,512] PSUM chunks, where the
     stationary operand is wc[:,1] read with a stride-0 (broadcast) access
     pattern — every output row i equals s_right, so the matmul doubles as
     the partition broadcast. ACT copies chunks into one SBUF tile
     [128, 2048] f32.
     PE: s_left column per 128-row block: xT_blk.T @ wc[:,0] -> PSUM
     [128,16]; ACT folds bias b while copying to SBUF.
  3. Main loop over 8 row-block pairs (BPI=2):
       adj_t <- DMA 1MB bf16 block
       att_t <- ACT sigmoid(sr_b + bias=s_left[i]) -> bf16
       out_t <- DVE att_t * adj_t (all-bf16 tensor_tensor: 2x DVE mode)
       DMA out 1MB bf16 block
  4. DMA issue order on the SP queue: xt, adj0 (its DGE generation
     pipelines under the xt transfer), bb, wc, adj1..7 — all 8 adj loads
     hoisted before the loop body so the whole input stream is generated
     up-front (adj_bufs=8 holds the full 8MB); the out issues that follow
     block SP on their multiply, but nothing queues behind them. ACT does
     only compute. The DMA stream is gap-free from ~2us to the end; the
     makespan is startup (~2us) + total DMA bytes / 360GB/s (~48.2us) +
     drain (~1.6us).

repeat / repeat_full are timing-only knobs (loop amplification for
wall-clock delta measurements); production uses both = 1.
"""

from contextlib import ExitStack

import numpy as np

import concourse.bass as bass
import concourse.tile as tile
from concourse import bacc, mybir
from concourse.bass_utils import run_bass_kernel_spmd

N = 2048
D = 128
P = 128
NBLK = N // P  # 16
NCORES = 8

_cache = {}


def _build(
    adj_bufs=8,
    att_bufs=8,
    out_bufs=5,
    blocks_per_iter=2,
    repeat=1,  # timing: main loop only, repeated
    warm_act=True,
    io_bf16=True,  # adj in / out as bfloat16 (halves HBM traffic)
    att_bf16=True,  # att tile bf16 -> all-bf16 DVE multiply (fast mode)
    repeat_full=1,  # timing: chain the ENTIRE kernel (setup included) M times
) -> bass.Bass:
    f32 = mybir.dt.float32
    # x and W move and multiply as fp16: same 2 bytes/elem as bf16 (half
    # the xt DMA vs f32, PE full rate) but 11-bit significand, so the
    # s_left/s_right dot-product error is negligible next to the bf16
    # adj/att/out roundings. x ~ N(0,1) and |W| < 0.0625 sit safely inside
    # fp16 normal range. (adj itself must stay bf16: fp16 denormals below
    # 6e-5 would blow up the max-relative-error metric.)
    f16 = mybir.dt.float16
    bf16 = mybir.dt.bfloat16
    dt_io = bf16 if io_bf16 else f32
    dt_att = bf16 if att_bf16 else f32
    nc = bacc.Bacc("TRN2", target_bir_lowering=False, debug=False)

    xt = nc.dram_tensor("xt", [D, N], f16, kind="ExternalInput").ap()
    adj = nc.dram_tensor("adj", [N, N], dt_io, kind="ExternalInput").ap()
    bb = nc.dram_tensor("bb", [P, 1], f32, kind="ExternalInput").ap()
    wc = nc.dram_tensor("wc", [P, 2], f16, kind="ExternalInput").ap()
    out = nc.dram_tensor("out", [N, N], dt_io, kind="ExternalOutput").ap()

    with ExitStack() as ctx:
        tc = ctx.enter_context(tile.TileContext(nc))
        const = ctx.enter_context(tc.tile_pool(name="const", bufs=1))
        rot = ctx.enter_context(
            tc.tile_pool(name="rot", bufs=2 if repeat_full > 1 else 1)
        )
        adj_pool = ctx.enter_context(tc.tile_pool(name="adjp", bufs=adj_bufs))
        att_pool = ctx.enter_context(tc.tile_pool(name="attp", bufs=att_bufs))
        out_pool = ctx.enter_context(tc.tile_pool(name="outp", bufs=out_bufs))
        srp_pool = ctx.enter_context(tc.tile_pool(name="srp", bufs=4, space="PSUM"))
        slp_pool = ctx.enter_context(tc.tile_pool(name="slp", bufs=2, space="PSUM"))

        BPI = blocks_per_iter
        W_ = N * BPI
        NIT = NBLK // BPI

        def load_adj(it):
            i0 = it * BPI
            adj_t = adj_pool.tile([P, W_], dt_io)
            if BPI == 1:
                nc.sync.dma_start(adj_t[:], adj[i0 * P : (i0 + 1) * P, :])
            else:
                nc.sync.dma_start(
                    adj_t[:].rearrange("p (u j) -> p u j", u=BPI),
                    adj[i0 * P : (i0 + BPI) * P, :].rearrange(
                        "(u p) j -> p u j", p=P
                    ),
                )
            return adj_t

        hoist = repeat == 1 and adj_bufs >= NIT

        # Setup-critical DMAs first: xt feeds every PE op. adj0's issue goes
        # second so its descriptor generation pipelines under the xt
        # transfer and the adj stream starts with zero gap after xt/consts.
        xt_t = const.tile([P, N], f16)
        nc.sync.dma_start(xt_t[:], xt)
        adj0 = load_adj(0) if hoist and repeat_full == 1 else None
        bb_t = const.tile([P, 1], f32)
        nc.sync.dma_start(bb_t[:], bb)
        wc_t = const.tile([P, 2], f16)
        nc.sync.dma_start(wc_t[:], wc)

        if warm_act:
            # Load the sigmoid ACT table at t=0, off the critical path.
            warm = const.tile([P, 1], f32)
            nc.vector.memset(warm[:], 0.0)
            nc.scalar.activation(
                warm[:], warm[:], mybir.ActivationFunctionType.Sigmoid
            )

        for _rep in range(repeat_full):
            # sr_b[i, j] = s_right[j] for every partition i: one matmul per
            # 512-col chunk with w_r (stride-0 broadcast of wc[:,1]) in every
            # stationary column — the matmul doubles as partition broadcast.
            sr_b = rot.tile([P, N], f32, tag="srb")
            wr_bc = wc_t[:, 1:2].broadcast_to([P, P])
            for c in range(4):
                src = srp_pool.tile([P, 512], f32)
                nc.tensor.matmul(
                    src[:], wr_bc, xt_t[:, c * 512 : (c + 1) * 512]
                )
                nc.scalar.copy(sr_b[:, c * 512 : (c + 1) * 512], src[:])

            # s_left columns [128, 16], bias b folded in the ACT copy.
            sl_t = rot.tile([P, NBLK], f32, tag="sl")
            sl_ps = slp_pool.tile([P, NBLK], f32)
            for i in range(NBLK):
                nc.tensor.matmul(
                    sl_ps[:, i : i + 1],
                    xt_t[:, i * P : (i + 1) * P],
                    wc_t[:, 0:1],
                )
            nc.scalar.add(sl_t[:], sl_ps[:], bb_t[:, 0:1])

            # --- main loop over row blocks ---
            # Hoist every adj load before the loop body: all NIT issues sit
            # at the head of the SP queue with no deps, so the whole input
            # stream is generated up-front; the out issues that follow block
            # SP on their multiply, but nothing queues behind them.
            hoisted = None
            if hoist:
                first = (
                    [adj0] if (adj0 is not None and _rep == 0) else [load_adj(0)]
                )
                hoisted = first + [load_adj(it) for it in range(1, NIT)]

            for it_rep in range(repeat * NIT):
                it = it_rep % NIT
                i0 = it * BPI
                adj_t = hoisted[it] if hoisted is not None else load_adj(it)
                att_t = att_pool.tile([P, W_], dt_att)
                for u in range(BPI):
                    nc.scalar.activation(
                        att_t[:, u * N : (u + 1) * N],
                        sr_b[:],
                        mybir.ActivationFunctionType.Sigmoid,
                        bias=sl_t[:, i0 + u : i0 + u + 1],
                        scale=1.0,
                    )
                o_t = out_pool.tile([P, W_], dt_io)
                nc.vector.tensor_tensor(
                    o_t[:], att_t[:], adj_t[:], op=mybir.AluOpType.mult
                )
                if BPI == 1:
                    nc.sync.dma_start(out[i0 * P : (i0 + 1) * P, :], o_t[:])
                else:
                    nc.sync.dma_start(
                        out[i0 * P : (i0 + BPI) * P, :].rearrange(
                            "(u p) j -> p u j", p=P
                        ),
                        o_t[:].rearrange("p (u j) -> p u j", u=BPI),
                    )

    nc.compile()
    return nc


PROD_CONFIG = dict(
    adj_bufs=8,
    att_bufs=8,
    out_bufs=5,
    blocks_per_iter=2,
    io_bf16=True,
    att_bf16=True,
)


def _get_nc() -> bass.Bass:
    if "nc" not in _cache:
        _cache["nc"] = _build(**PROD_CONFIG)
    return _cache["nc"]


def _io_dtypes(nc):
    """(adj_np_dtype, out_np_dtype) as declared by the module."""
    import concourse.mybir as _mb

    adj_dt = out_dt = np.float32
    for alloc in nc.m.functions[0].allocations:
        if isinstance(alloc, _mb.MemoryLocationSet):
            if alloc.kind == "ExternalInput" and (
                alloc.memorylocations[0].name == "adj"
            ):
                adj_dt = mybir.dt.np(alloc.dtype)
            if alloc.kind == "ExternalOutput" and (
                alloc.memorylocations[0].name == "out"
            ):
                out_dt = mybir.dt.np(alloc.dtype)
    return adj_dt, out_dt


def _in_maps(x, adj, W, b, nc=None):
    import ml_dtypes

    if nc is None:
        nc = _get_nc()
    adj_dt, _ = _io_dtypes(nc)
    bf = ml_dtypes.bfloat16
    x = np.asarray(x, dtype=np.float32)
    adj = np.ascontiguousarray(np.asarray(adj, dtype=np.float32).astype(adj_dt))
    W = np.asarray(W, dtype=np.float32)
    b = np.float32(np.asarray(b, dtype=np.float32))
    shared = {
        "wc": np.ascontiguousarray(W.reshape(2, D).T.astype(np.float16)),
        "bb": np.full((P, 1), b, dtype=np.float32),
    }
    return [
        {"xt": np.ascontiguousarray(x[c].T.astype(np.float16)), "adj": adj[c], **shared}
        for c in range(NCORES)
    ]


def run(x, adj, W, b, trace=False):
    import os

    if not trace:
        # This axon client image has no NTFF profile hook
        # (antenv.axon_hooks); an inherited BASS_TRACE=1 would crash the
        # run on that import, so force tracing off.
        os.environ["BASS_NEVER_TRACE"] = "1"
    nc = _get_nc()
    in_maps = _in_maps(x, adj, W, b, nc=nc)
    res = None
    last_err = None
    for attempt in range(3):
        try:
            res = run_bass_kernel_spmd(
                nc,
                in_maps,
                core_ids=list(range(NCORES)),
                trace=trace,
            )
            break
        except Exception as e:  # transient device wedge (NRT_EXEC_UNIT_...)
            last_err = e
            import time

            time.sleep(2.0 * (attempt + 1))
    if res is None:
        raise last_err
    out = np.stack(
        [np.asarray(res.results[c]["out"]) for c in range(NCORES)], axis=0
    ).astype(np.float32)
    return out, res


def kernel(x, adj, W, b):
    out, _ = run(x, adj, W, b)
    return out
